# revision 2
# baseline (speedup 1.0000x reference)
"""MoE routing model (nn_MoEModel) as a Bass/Tile kernel for 8 Trainium2 cores.

Strategy (pure batch data-parallelism, feature-major on device):
  - x is host-transposed to [65, B] (row 64 = ones so b1 rides in the L1 matmul).
  - Per core: stream 512-row macro-tiles through
        z1 = relu(x @ W1 + b1)        2x fp32r matmul [65,128] x [65,512]
        z  = z1 @ W2 (+ b2 folded)    2x accumulating matmul
        h  = relu(z @ Wr1 + br1')     experts packed in pairs -> 3x [128,128]
                                      + 1x [128,71] (expert6 | gate) matmul
        preds = h . Wr2 (blockdiag)   4x accumulating [*,7] matmuls
        expw  = exp(gate logits)      ACT Exp
    Device returns preds [7,B] and expw [7,B]; the O(B*7) tail (softmax
    normalization, y_soft reduction, y_hard gather, +br2) runs in numpy.
  - All matmuls use float32r (fp32 bits, relaxed PE mode, 1 cyc/row at N=512).
"""

import os
import sys

sys.path.insert(0, "/opt/trn_rl_repo")

import numpy as np

import concourse.bass as bass  # noqa: E402
import concourse.tile as tile  # noqa: E402
from concourse import bacc, mybir  # noqa: E402
from concourse.bass_utils import run_bass_kernel_spmd  # noqa: E402

NCORES = 8
B = 262144
BC = B // NCORES  # 32768 rows per core
IN, H, F, K, RH = 64, 256, 128, 7, 64
NB = 512  # macro-tile batch (matmul moving free dim)
NG = 4  # macro-tiles per x-load group
GB = NB * NG  # 2048

AF = mybir.ActivationFunctionType
ALU = mybir.AluOpType
F32 = mybir.dt.float32
F32R = mybir.dt.float32r

# set by test.py to capture profiling info
TRACE = False
LAST_RESULTS = None

_module_cache = {}


def _build_module(bc: int):
    """Trace + compile the per-core Bass module for a batch slice of bc rows."""
    if bc in _module_cache:
        return _module_cache[bc]

    nc = bacc.Bacc(
        "TRN2",
        target_bir_lowering=False,
        debug=False,
        enable_asserts=False,
        num_devices=NCORES,
    )

    xT = nc.dram_tensor("xT", [IN + 1, bc], F32R, kind="ExternalInput").ap()
    w1a_d = nc.dram_tensor("w1a", [IN + 1, 128], F32R, kind="ExternalInput").ap()
    w1b_d = nc.dram_tensor("w1b", [IN + 1, 128], F32R, kind="ExternalInput").ap()
    w2a_d = nc.dram_tensor("w2a", [128, 128], F32R, kind="ExternalInput").ap()
    w2b_d = nc.dram_tensor("w2b", [128, 128], F32R, kind="ExternalInput").ap()
    wp_d = [
        nc.dram_tensor("wp0", [128, 128], F32R, kind="ExternalInput").ap(),
        nc.dram_tensor("wp1", [128, 128], F32R, kind="ExternalInput").ap(),
        nc.dram_tensor("wp2", [128, 128], F32R, kind="ExternalInput").ap(),
        nc.dram_tensor("wp3", [128, 64 + K], F32R, kind="ExternalInput").ap(),
    ]
    wb_d = [
        nc.dram_tensor("wb0", [128, K], F32R, kind="ExternalInput").ap(),
        nc.dram_tensor("wb1", [128, K], F32R, kind="ExternalInput").ap(),
        nc.dram_tensor("wb2", [128, K], F32R, kind="ExternalInput").ap(),
        nc.dram_tensor("wb3", [64, K], F32R, kind="ExternalInput").ap(),
    ]
    brp_d = [
        nc.dram_tensor("brp0", [128, 1], F32, kind="ExternalInput").ap(),
        nc.dram_tensor("brp1", [128, 1], F32, kind="ExternalInput").ap(),
        nc.dram_tensor("brp2", [128, 1], F32, kind="ExternalInput").ap(),
        nc.dram_tensor("brp3", [64, 1], F32, kind="ExternalInput").ap(),
    ]
    bg_d = nc.dram_tensor("bg", [K, 1], F32, kind="ExternalInput").ap()

    preds_out = nc.dram_tensor("preds_out", [K, bc], F32, kind="ExternalOutput").ap()
    expw_out = nc.dram_tensor("expw_out", [K, bc], F32, kind="ExternalOutput").ap()

    ngroups = bc // GB

    from contextlib import ExitStack

    with tile.TileContext(nc) as tc, ExitStack() as ctx:
        consts = ctx.enter_context(tc.tile_pool(name="consts", bufs=1))
        xpool = ctx.enter_context(tc.tile_pool(name="xpool", bufs=2))
        work = ctx.enter_context(tc.tile_pool(name="work", bufs=2))
        opool = ctx.enter_context(tc.tile_pool(name="opool", bufs=2))
        psum = ctx.enter_context(tc.tile_pool(name="psum", bufs=1, space="PSUM"))

        def load_const(dram_ap, cname, shape, dt=F32R):
            t = consts.tile(shape, dt, name=cname, tag=cname)
            nc.sync.dma_start(t[:], dram_ap)
            return t

        w1a_s = load_const(w1a_d, "w1a_s", [IN + 1, 128])
        w1b_s = load_const(w1b_d, "w1b_s", [IN + 1, 128])
        w2a_s = load_const(w2a_d, "w2a_s", [128, 128])
        w2b_s = load_const(w2b_d, "w2b_s", [128, 128])
        wp_s = [
            load_const(wp_d[0], "wp0_s", [128, 128]),
            load_const(wp_d[1], "wp1_s", [128, 128]),
            load_const(wp_d[2], "wp2_s", [128, 128]),
            load_const(wp_d[3], "wp3_s", [128, 64 + K]),
        ]
        wb_s = [
            load_const(wb_d[0], "wb0_s", [128, K]),
            load_const(wb_d[1], "wb1_s", [128, K]),
            load_const(wb_d[2], "wb2_s", [128, K]),
            load_const(wb_d[3], "wb3_s", [64, K]),
        ]
        brp_s = [
            load_const(brp_d[0], "brp0_s", [128, 1], F32),
            load_const(brp_d[1], "brp1_s", [128, 1], F32),
            load_const(brp_d[2], "brp2_s", [128, 1], F32),
            load_const(brp_d[3], "brp3_s", [64, 1], F32),
        ]
        # gate bias lives on partitions 64..70 to line up with the gate rows
        # of the (expert6 | gate) psum tile
        bg_s = consts.tile([64 + K, 1], F32, name="bg_s", tag="bg_s")
        nc.sync.dma_start(bg_s[64 : 64 + K, :], bg_d)

        for g in range(ngroups):
            xt = xpool.tile([IN + 1, GB], F32R, name=f"xt{g}", tag="xt")
            nc.sync.dma_start(xt[:], xT[:, g * GB : (g + 1) * GB])
            predss = opool.tile([K, GB], F32, name=f"predss{g}", tag="predss")
            expws = opool.tile([64 + K, GB], F32, name=f"expws{g}", tag="expws")

            for j in range(NG):
                js = slice(j * NB, (j + 1) * NB)
                rhs_x = xt[:, js]

                # ---- extractor layer 1: z1 = relu(x @ W1 + b1), [256, NB]
                z1p = psum.tile([128, 2 * NB], F32, name=f"z1p_{g}_{j}", tag="z1p")
                nc.tensor.matmul(
                    z1p[:, 0:NB], w1a_s[:], rhs_x, start=True, stop=True
                )
                nc.tensor.matmul(
                    z1p[:, NB:], w1b_s[:], rhs_x, start=True, stop=True
                )
                z1s = work.tile([128, 2 * NB], F32R, name=f"z1s_{g}_{j}", tag="z1s")
                nc.scalar.activation(z1s[:], z1p[:], AF.Relu)

                # ---- extractor layer 2: z = z1 @ W2 (b2 folded downstream)
                zp = psum.tile([128, NB], F32, name=f"zp_{g}_{j}", tag="zp")
                nc.tensor.matmul(
                    zp[:], w2a_s[:], z1s[:, 0:NB], start=True, stop=False
                )
                nc.tensor.matmul(
                    zp[:], w2b_s[:], z1s[:, NB:], start=False, stop=True
                )
                zs = work.tile([128, NB], F32R, name=f"zs_{g}_{j}", tag="zs")
                nc.vector.tensor_copy(zs[:], zp[:])

                # ---- expert hidden layers (pairs) + gate logits
                hp = []
                for p in range(3):
                    hpp = psum.tile([128, NB], F32, name=f"hp{p}_{g}_{j}", tag=f"hp{p}")
                    nc.tensor.matmul(
                        hpp[:], wp_s[p][:], zs[:], start=True, stop=True
                    )
                    hp.append(hpp)
                hpg = psum.tile([64 + K, NB], F32, name=f"hpg_{g}_{j}", tag="hpg")
                nc.tensor.matmul(hpg[:], wp_s[3][:], zs[:], start=True, stop=True)

                h0s = work.tile([128, NB], F32R, name=f"h0s_{g}_{j}", tag="h0s")
                nc.scalar.activation(h0s[:], hp[0][:], AF.Relu, bias=brp_s[0][:])
                h1s = work.tile([128, NB], F32R, name=f"h1s_{g}_{j}", tag="h1s")
                nc.scalar.activation(h1s[:], hp[1][:], AF.Relu, bias=brp_s[1][:])
                h2s = work.tile([128, NB], F32R, name=f"h2s_{g}_{j}", tag="h2s")
                nc.vector.tensor_scalar(
                    h2s[:], hp[2][:], brp_s[2][:], 0.0, ALU.add, ALU.max
                )
                h6s = work.tile([64, NB], F32R, name=f"h6s_{g}_{j}", tag="h6s")
                nc.vector.tensor_scalar(
                    h6s[:], hpg[0:64, :], brp_s[3][:], 0.0, ALU.add, ALU.max
                )
                # gate: expw = exp(logits + bg)
                nc.scalar.activation(
                    expws[64 : 64 + K, js],
                    hpg[64 : 64 + K, :],
                    AF.Exp,
                    bias=bg_s[64 : 64 + K, :],
                )

                # ---- expert heads: preds[k] = h[k] . Wr2[k]  (blockdiag accum)
                pp = psum.tile([K, NB], F32, name=f"pp_{g}_{j}", tag="pp")
                nc.tensor.matmul(pp[:], wb_s[0][:], h0s[:], start=True, stop=False)
                nc.tensor.matmul(
                    pp[:], wb_s[1][:], h1s[:], start=False, stop=False
                )
                nc.tensor.matmul(
                    pp[:], wb_s[2][:], h2s[:], start=False, stop=False
                )
                nc.tensor.matmul(pp[:], wb_s[3][:], h6s[:], start=False, stop=True)
                nc.vector.tensor_copy(predss[:, js], pp[:])

            nc.sync.dma_start(preds_out[:, g * GB : (g + 1) * GB], predss[:])
            nc.sync.dma_start(expw_out[:, g * GB : (g + 1) * GB], expws[64 : 64 + K, :])

    nc.compile()
    _module_cache[bc] = nc
    return nc


def _prep_shared(W1, b1, W2, b2, Wr1, br1, Wr2, br2, Wg, bg):
    """Host-side packing of the (tiny, replicated) weights."""
    f = np.float32
    W1 = np.asarray(W1, f)
    b1 = np.asarray(b1, f)
    W2 = np.asarray(W2, f)
    b2 = np.asarray(b2, f)
    Wr1 = np.asarray(Wr1, f)
    br1 = np.asarray(br1, f)
    Wr2 = np.asarray(Wr2, f)
    Wg = np.asarray(Wg, f)
    bg = np.asarray(bg, f)

    w1c = np.vstack([W1, b1[None, :]])  # [65, 256]
    out = {
        "w1a": np.ascontiguousarray(w1c[:, :128]),
        "w1b": np.ascontiguousarray(w1c[:, 128:]),
        "w2a": np.ascontiguousarray(W2[:128]),
        "w2b": np.ascontiguousarray(W2[128:]),
        "wp0": np.ascontiguousarray(np.concatenate([Wr1[0], Wr1[1]], 1)),
        "wp1": np.ascontiguousarray(np.concatenate([Wr1[2], Wr1[3]], 1)),
        "wp2": np.ascontiguousarray(np.concatenate([Wr1[4], Wr1[5]], 1)),
        "wp3": np.ascontiguousarray(np.concatenate([Wr1[6], Wg], 1)),  # [128, 71]
    }
    # fold b2 into the expert/gate input biases: h = relu(z~ @ Wr1 + br1') etc.
    br1_eff = br1 + np.einsum("f,kfh->kh", b2, Wr1)  # [K, RH]
    bg_eff = (bg + b2 @ Wg).astype(f)  # [K]
    out["brp0"] = np.concatenate([br1_eff[0], br1_eff[1]])[:, None].astype(f)
    out["brp1"] = np.concatenate([br1_eff[2], br1_eff[3]])[:, None].astype(f)
    out["brp2"] = np.concatenate([br1_eff[4], br1_eff[5]])[:, None].astype(f)
    out["brp3"] = np.ascontiguousarray(br1_eff[6][:, None]).astype(f)
    out["bg"] = bg_eff[:, None]

    for p in range(3):
        wb = np.zeros((128, K), f)
        wb[0:64, 2 * p] = Wr2[2 * p, :, 0]
        wb[64:128, 2 * p + 1] = Wr2[2 * p + 1, :, 0]
        out[f"wb{p}"] = wb
    wb3 = np.zeros((64, K), f)
    wb3[:, 6] = Wr2[6, :, 0]
    out["wb3"] = wb3
    return out


def kernel(x, domain, W1, b1, W2, b2, Wr1, br1, Wr2, br2, Wg, bg):
    global LAST_RESULTS
    x = np.asarray(x, np.float32)
    domain = np.asarray(domain)
    br2 = np.asarray(br2, np.float32)

    nc = _build_module(BC)
    shared = _prep_shared(W1, b1, W2, b2, Wr1, br1, Wr2, br2, Wg, bg)

    xT = np.empty((IN + 1, B), np.float32)
    xT[:IN] = x.T
    xT[IN] = 1.0

    in_maps = []
    for i in range(NCORES):
        m = dict(shared)
        m["xT"] = np.ascontiguousarray(xT[:, i * BC : (i + 1) * BC])
        in_maps.append(m)

    res = run_bass_kernel_spmd(
        nc, in_maps, core_ids=list(range(NCORES)), trace=TRACE
    )
    LAST_RESULTS = res

    preds = np.concatenate(
        [np.asarray(r["preds_out"]).T for r in res.results], 0
    )  # [B, K]
    expw = np.concatenate([np.asarray(r["expw_out"]).T for r in res.results], 0)

    preds = preds + br2[:, 0][None, :]
    idx = np.clip(domain.astype(np.int64) - 1, 0, K - 1)
    y_hard = np.take_along_axis(preds, idx[:, None], axis=1)
    s = expw.sum(1, keepdims=True)
    weights = expw / s
    y_soft = (weights * preds).sum(1, keepdims=True)
    return (
        y_hard.astype(np.float32),
        y_soft.astype(np.float32),
        weights.astype(np.float32),
    )


# revision 6
# speedup vs baseline: 15.2282x; 15.2282x over previous
"""MoE routing model (nn_MoEModel) as a Bass/Tile kernel for 8 Trainium2 cores.

Strategy (pure batch data-parallelism, feature-major on device):
  - x is host-transposed to [65, B] (row 64 = ones so b1 rides in the L1 matmul).
  - Per core: stream 512-row macro-tiles through
        z1 = relu(x @ W1 + b1)        2x fp32r matmul [65,128] x [65,512]
        z  = z1 @ W2 (+ b2 folded)    2x accumulating matmul
        h  = relu(z @ Wr1 + br1')     experts packed in pairs -> 3x [128,128]
                                      + 1x [128,71] (expert6 | gate) matmul
        preds = h . Wr2 (blockdiag)   4x accumulating [*,7] matmuls
        expw  = exp(gate logits)      ACT Exp
    Device returns preds [7,B] and expw [7,B]; the O(B*7) tail (softmax
    normalization, y_soft reduction, y_hard gather, +br2) runs in numpy.
  - All matmuls use float32r (fp32 bits, relaxed PE mode, 1 cyc/row at N=512).
"""

import os
import sys

sys.path.insert(0, "/opt/trn_rl_repo")

import numpy as np

import concourse.bass as bass  # noqa: E402
import concourse.tile as tile  # noqa: E402
from concourse import bacc, mybir  # noqa: E402
from concourse.bass_utils import run_bass_kernel_spmd  # noqa: E402

NCORES = 8
B = 262144
BC = B // NCORES  # 32768 rows per core
IN, H, F, K, RH = 64, 256, 128, 7, 64
NB = 512  # macro-tile batch (matmul moving free dim)
NG = 4  # macro-tiles per x-load group
GB = NB * NG  # 2048

AF = mybir.ActivationFunctionType
ALU = mybir.AluOpType
F32 = mybir.dt.float32
F32R = mybir.dt.float32r

# set by test.py to capture profiling info
TRACE = False
LAST_RESULTS = None

_module_cache = {}


def _build_fast_module(bc: int):
    """Zero-bias fast path (the shipped problem has all-zero biases).

    Differences vs the general module:
      - no ones-row / bias tiles at all
      - L1 runs as two concurrent PE row-group matmuls (K=64 each, x
        duplicated into both partition halves of the x tile)
      - all four expert matmuls write one [128, 2048] psum region, evacuated
        by one big relu (pairs), one h6 relu and one exp — fewer, larger
        elementwise ops
    """
    key = ("fast", bc)
    if key in _module_cache:
        return _module_cache[key]

    nc = bacc.Bacc(
        "TRN2",
        target_bir_lowering=False,
        debug=False,
        enable_asserts=False,
        num_devices=NCORES,
    )

    xT = nc.dram_tensor("xT", [IN, bc], F32R, kind="ExternalInput").ap()
    w1ab_d = nc.dram_tensor("w1ab", [128, 128], F32R, kind="ExternalInput").ap()
    w2a_d = nc.dram_tensor("w2a", [128, 128], F32R, kind="ExternalInput").ap()
    w2b_d = nc.dram_tensor("w2b", [128, 128], F32R, kind="ExternalInput").ap()
    wp_d = [
        nc.dram_tensor("wp0", [128, 128], F32R, kind="ExternalInput").ap(),
        nc.dram_tensor("wp1", [128, 128], F32R, kind="ExternalInput").ap(),
        nc.dram_tensor("wp2", [128, 128], F32R, kind="ExternalInput").ap(),
        nc.dram_tensor("wp3", [128, 64 + K], F32R, kind="ExternalInput").ap(),
    ]
    wb_d = [
        nc.dram_tensor("wb0", [128, K], F32R, kind="ExternalInput").ap(),
        nc.dram_tensor("wb1", [128, K], F32R, kind="ExternalInput").ap(),
        nc.dram_tensor("wb2", [128, K], F32R, kind="ExternalInput").ap(),
        nc.dram_tensor("wb3", [64, K], F32R, kind="ExternalInput").ap(),
    ]
    preds_out = nc.dram_tensor("preds_out", [K, bc], F32, kind="ExternalOutput").ap()
    expw_out = nc.dram_tensor("expw_out", [K, bc], F32, kind="ExternalOutput").ap()

    ngroups = bc // GB

    from contextlib import ExitStack

    with tile.TileContext(nc) as tc, ExitStack() as ctx:
        consts = ctx.enter_context(tc.tile_pool(name="consts", bufs=1))
        xpool = ctx.enter_context(tc.tile_pool(name="xpool", bufs=2))
        work = ctx.enter_context(tc.tile_pool(name="work", bufs=2))
        opool = ctx.enter_context(tc.tile_pool(name="opool", bufs=2))
        psum = ctx.enter_context(tc.tile_pool(name="psum", bufs=1, space="PSUM"))

        def load_const(dram_ap, cname, shape):
            t = consts.tile(shape, F32R, name=cname, tag=cname)
            nc.sync.dma_start(t[:], dram_ap)
            return t

        w1ab_s = load_const(w1ab_d, "w1ab_s", [128, 128])
        w2a_s = load_const(w2a_d, "w2a_s", [128, 128])
        w2b_s = load_const(w2b_d, "w2b_s", [128, 128])
        wp_s = [
            load_const(wp_d[0], "wp0_s", [128, 128]),
            load_const(wp_d[1], "wp1_s", [128, 128]),
            load_const(wp_d[2], "wp2_s", [128, 128]),
            load_const(wp_d[3], "wp3_s", [128, 64 + K]),
        ]
        wb_s = [
            load_const(wb_d[0], "wb0_s", [128, K]),
            load_const(wb_d[1], "wb1_s", [128, K]),
            load_const(wb_d[2], "wb2_s", [128, K]),
            load_const(wb_d[3], "wb3_s", [64, K]),
        ]

        for g in range(ngroups):
            gs = slice(g * GB, (g + 1) * GB)
            # x duplicated into both partition halves (feeds the two
            # concurrent L1 row-group matmuls)
            xt = xpool.tile([128, GB], F32R, name=f"xt{g}", tag="xt")
            nc.sync.dma_start(xt[0:IN, :], xT[:, gs])
            nc.sync.dma_start(xt[IN:128, :], xT[:, gs])
            predss = opool.tile([K, GB], F32, name=f"predss{g}", tag="predss")
            expws = opool.tile([64 + K, GB], F32, name=f"expws{g}", tag="expws")

            for j in range(NG):
                js = slice(j * NB, (j + 1) * NB)

                # ---- L1: two concurrent row-group matmuls (K=64 each)
                z1p = psum.tile([128, 2 * NB], F32, name=f"z1p_{g}_{j}", tag="z1p")
                nc.tensor.matmul(
                    z1p[:, 0:NB], w1ab_s[0:IN, :], xt[0:IN, js], start=True, stop=True
                )
                nc.tensor.matmul(
                    z1p[:, NB:], w1ab_s[IN:128, :], xt[IN:128, js], start=True, stop=True
                )
                z1s = work.tile([128, 2 * NB], F32R, name=f"z1s_{g}_{j}", tag="z1s")
                nc.vector.tensor_scalar(z1s[:], z1p[:], 0.0, None, ALU.max)

                # ---- L2
                zp = psum.tile([128, NB], F32, name=f"zp_{g}_{j}", tag="zp")
                nc.tensor.matmul(zp[:], w2a_s[:], z1s[:, 0:NB], start=True, stop=False)
                nc.tensor.matmul(zp[:], w2b_s[:], z1s[:, NB:], start=False, stop=True)
                zs = work.tile([128, NB], F32R, name=f"zs_{g}_{j}", tag="zs")
                nc.vector.tensor_copy(zs[:], zp[:])

                # ---- experts + gate into ONE 4-bank psum region
                hbig = psum.tile([128, 4 * NB], F32, name=f"hbig_{g}_{j}", tag="hbig")
                for p in range(3):
                    nc.tensor.matmul(
                        hbig[:, p * NB : (p + 1) * NB],
                        wp_s[p][:],
                        zs[:],
                        start=True,
                        stop=True,
                    )
                nc.tensor.matmul(
                    hbig[0 : 64 + K, 3 * NB :], wp_s[3][:], zs[:], start=True, stop=True
                )

                h_all = work.tile([128, 4 * NB], F32R, name=f"h_{g}_{j}", tag="h_all")
                # pairs 0..2 in one big relu
                nc.scalar.activation(
                    h_all[:, 0 : 3 * NB], hbig[:, 0 : 3 * NB], AF.Relu
                )
                # expert 6 rows of the 4th bank
                nc.scalar.activation(
                    h_all[0:64, 3 * NB :], hbig[0:64, 3 * NB :], AF.Relu
                )
                # gate rows -> exp
                nc.scalar.activation(
                    expws[64 : 64 + K, js], hbig[64 : 64 + K, 3 * NB :], AF.Exp
                )

                # ---- expert heads (blockdiag accumulate)
                pp = psum.tile([K, NB], F32, name=f"pp_{g}_{j}", tag="pp")
                nc.tensor.matmul(
                    pp[:], wb_s[0][:], h_all[:, 0:NB], start=True, stop=False
                )
                nc.tensor.matmul(
                    pp[:], wb_s[1][:], h_all[:, NB : 2 * NB], start=False, stop=False
                )
                nc.tensor.matmul(
                    pp[:], wb_s[2][:], h_all[:, 2 * NB : 3 * NB], start=False, stop=False
                )
                nc.tensor.matmul(
                    pp[:], wb_s[3][:], h_all[0:64, 3 * NB :], start=False, stop=True
                )
                nc.vector.tensor_copy(predss[:, js], pp[:])

            nc.sync.dma_start(preds_out[:, gs], predss[:])
            nc.sync.dma_start(expw_out[:, gs], expws[64 : 64 + K, :])

    nc.compile()
    _module_cache[key] = nc
    return nc


def _build_module(bc: int):
    """Trace + compile the per-core Bass module for a batch slice of bc rows."""
    if bc in _module_cache:
        return _module_cache[bc]

    nc = bacc.Bacc(
        "TRN2",
        target_bir_lowering=False,
        debug=False,
        enable_asserts=False,
        num_devices=NCORES,
    )

    xT = nc.dram_tensor("xT", [IN + 1, bc], F32R, kind="ExternalInput").ap()
    w1a_d = nc.dram_tensor("w1a", [IN + 1, 128], F32R, kind="ExternalInput").ap()
    w1b_d = nc.dram_tensor("w1b", [IN + 1, 128], F32R, kind="ExternalInput").ap()
    w2a_d = nc.dram_tensor("w2a", [128, 128], F32R, kind="ExternalInput").ap()
    w2b_d = nc.dram_tensor("w2b", [128, 128], F32R, kind="ExternalInput").ap()
    wp_d = [
        nc.dram_tensor("wp0", [128, 128], F32R, kind="ExternalInput").ap(),
        nc.dram_tensor("wp1", [128, 128], F32R, kind="ExternalInput").ap(),
        nc.dram_tensor("wp2", [128, 128], F32R, kind="ExternalInput").ap(),
        nc.dram_tensor("wp3", [128, 64 + K], F32R, kind="ExternalInput").ap(),
    ]
    wb_d = [
        nc.dram_tensor("wb0", [128, K], F32R, kind="ExternalInput").ap(),
        nc.dram_tensor("wb1", [128, K], F32R, kind="ExternalInput").ap(),
        nc.dram_tensor("wb2", [128, K], F32R, kind="ExternalInput").ap(),
        nc.dram_tensor("wb3", [64, K], F32R, kind="ExternalInput").ap(),
    ]
    brp_d = [
        nc.dram_tensor("brp0", [128, 1], F32, kind="ExternalInput").ap(),
        nc.dram_tensor("brp1", [128, 1], F32, kind="ExternalInput").ap(),
        nc.dram_tensor("brp2", [128, 1], F32, kind="ExternalInput").ap(),
        nc.dram_tensor("brp3", [64, 1], F32, kind="ExternalInput").ap(),
    ]
    bg_d = nc.dram_tensor("bg", [K, 1], F32, kind="ExternalInput").ap()

    preds_out = nc.dram_tensor("preds_out", [K, bc], F32, kind="ExternalOutput").ap()
    expw_out = nc.dram_tensor("expw_out", [K, bc], F32, kind="ExternalOutput").ap()

    ngroups = bc // GB

    from contextlib import ExitStack

    with tile.TileContext(nc) as tc, ExitStack() as ctx:
        consts = ctx.enter_context(tc.tile_pool(name="consts", bufs=1))
        xpool = ctx.enter_context(tc.tile_pool(name="xpool", bufs=2))
        work = ctx.enter_context(tc.tile_pool(name="work", bufs=2))
        opool = ctx.enter_context(tc.tile_pool(name="opool", bufs=2))
        psum = ctx.enter_context(tc.tile_pool(name="psum", bufs=1, space="PSUM"))

        def load_const(dram_ap, cname, shape, dt=F32R):
            t = consts.tile(shape, dt, name=cname, tag=cname)
            nc.sync.dma_start(t[:], dram_ap)
            return t

        w1a_s = load_const(w1a_d, "w1a_s", [IN + 1, 128])
        w1b_s = load_const(w1b_d, "w1b_s", [IN + 1, 128])
        w2a_s = load_const(w2a_d, "w2a_s", [128, 128])
        w2b_s = load_const(w2b_d, "w2b_s", [128, 128])
        wp_s = [
            load_const(wp_d[0], "wp0_s", [128, 128]),
            load_const(wp_d[1], "wp1_s", [128, 128]),
            load_const(wp_d[2], "wp2_s", [128, 128]),
            load_const(wp_d[3], "wp3_s", [128, 64 + K]),
        ]
        wb_s = [
            load_const(wb_d[0], "wb0_s", [128, K]),
            load_const(wb_d[1], "wb1_s", [128, K]),
            load_const(wb_d[2], "wb2_s", [128, K]),
            load_const(wb_d[3], "wb3_s", [64, K]),
        ]
        brp_s = [
            load_const(brp_d[0], "brp0_s", [128, 1], F32),
            load_const(brp_d[1], "brp1_s", [128, 1], F32),
            load_const(brp_d[2], "brp2_s", [128, 1], F32),
            load_const(brp_d[3], "brp3_s", [64, 1], F32),
        ]
        # gate bias lives on partitions 64..70 to line up with the gate rows
        # of the (expert6 | gate) psum tile
        bg_s = consts.tile([64 + K, 1], F32, name="bg_s", tag="bg_s")
        nc.sync.dma_start(bg_s[64 : 64 + K, :], bg_d)

        for g in range(ngroups):
            xt = xpool.tile([IN + 1, GB], F32R, name=f"xt{g}", tag="xt")
            nc.sync.dma_start(xt[:], xT[:, g * GB : (g + 1) * GB])
            predss = opool.tile([K, GB], F32, name=f"predss{g}", tag="predss")
            expws = opool.tile([64 + K, GB], F32, name=f"expws{g}", tag="expws")

            for j in range(NG):
                js = slice(j * NB, (j + 1) * NB)
                rhs_x = xt[:, js]

                # ---- extractor layer 1: z1 = relu(x @ W1 + b1), [256, NB]
                z1p = psum.tile([128, 2 * NB], F32, name=f"z1p_{g}_{j}", tag="z1p")
                nc.tensor.matmul(
                    z1p[:, 0:NB], w1a_s[:], rhs_x, start=True, stop=True
                )
                nc.tensor.matmul(
                    z1p[:, NB:], w1b_s[:], rhs_x, start=True, stop=True
                )
                z1s = work.tile([128, 2 * NB], F32R, name=f"z1s_{g}_{j}", tag="z1s")
                nc.scalar.activation(z1s[:], z1p[:], AF.Relu)

                # ---- extractor layer 2: z = z1 @ W2 (b2 folded downstream)
                zp = psum.tile([128, NB], F32, name=f"zp_{g}_{j}", tag="zp")
                nc.tensor.matmul(
                    zp[:], w2a_s[:], z1s[:, 0:NB], start=True, stop=False
                )
                nc.tensor.matmul(
                    zp[:], w2b_s[:], z1s[:, NB:], start=False, stop=True
                )
                zs = work.tile([128, NB], F32R, name=f"zs_{g}_{j}", tag="zs")
                nc.vector.tensor_copy(zs[:], zp[:])

                # ---- expert hidden layers (pairs) + gate logits
                hp = []
                for p in range(3):
                    hpp = psum.tile([128, NB], F32, name=f"hp{p}_{g}_{j}", tag=f"hp{p}")
                    nc.tensor.matmul(
                        hpp[:], wp_s[p][:], zs[:], start=True, stop=True
                    )
                    hp.append(hpp)
                hpg = psum.tile([64 + K, NB], F32, name=f"hpg_{g}_{j}", tag="hpg")
                nc.tensor.matmul(hpg[:], wp_s[3][:], zs[:], start=True, stop=True)

                h0s = work.tile([128, NB], F32R, name=f"h0s_{g}_{j}", tag="h0s")
                nc.scalar.activation(h0s[:], hp[0][:], AF.Relu, bias=brp_s[0][:])
                h1s = work.tile([128, NB], F32R, name=f"h1s_{g}_{j}", tag="h1s")
                nc.scalar.activation(h1s[:], hp[1][:], AF.Relu, bias=brp_s[1][:])
                h2s = work.tile([128, NB], F32R, name=f"h2s_{g}_{j}", tag="h2s")
                nc.vector.tensor_scalar(
                    h2s[:], hp[2][:], brp_s[2][:], 0.0, ALU.add, ALU.max
                )
                h6s = work.tile([64, NB], F32R, name=f"h6s_{g}_{j}", tag="h6s")
                nc.vector.tensor_scalar(
                    h6s[:], hpg[0:64, :], brp_s[3][:], 0.0, ALU.add, ALU.max
                )
                # gate: expw = exp(logits + bg)
                nc.scalar.activation(
                    expws[64 : 64 + K, js],
                    hpg[64 : 64 + K, :],
                    AF.Exp,
                    bias=bg_s[64 : 64 + K, :],
                )

                # ---- expert heads: preds[k] = h[k] . Wr2[k]  (blockdiag accum)
                pp = psum.tile([K, NB], F32, name=f"pp_{g}_{j}", tag="pp")
                nc.tensor.matmul(pp[:], wb_s[0][:], h0s[:], start=True, stop=False)
                nc.tensor.matmul(
                    pp[:], wb_s[1][:], h1s[:], start=False, stop=False
                )
                nc.tensor.matmul(
                    pp[:], wb_s[2][:], h2s[:], start=False, stop=False
                )
                nc.tensor.matmul(pp[:], wb_s[3][:], h6s[:], start=False, stop=True)
                nc.vector.tensor_copy(predss[:, js], pp[:])

            nc.sync.dma_start(preds_out[:, g * GB : (g + 1) * GB], predss[:])
            nc.sync.dma_start(expw_out[:, g * GB : (g + 1) * GB], expws[64 : 64 + K, :])

    nc.compile()
    _module_cache[bc] = nc
    return nc


def _prep_shared(W1, b1, W2, b2, Wr1, br1, Wr2, br2, Wg, bg):
    """Host-side packing of the (tiny, replicated) weights."""
    f = np.float32
    W1 = np.asarray(W1, f)
    b1 = np.asarray(b1, f)
    W2 = np.asarray(W2, f)
    b2 = np.asarray(b2, f)
    Wr1 = np.asarray(Wr1, f)
    br1 = np.asarray(br1, f)
    Wr2 = np.asarray(Wr2, f)
    Wg = np.asarray(Wg, f)
    bg = np.asarray(bg, f)

    w1c = np.vstack([W1, b1[None, :]])  # [65, 256]
    out = {
        "w1a": np.ascontiguousarray(w1c[:, :128]),
        "w1b": np.ascontiguousarray(w1c[:, 128:]),
        "w2a": np.ascontiguousarray(W2[:128]),
        "w2b": np.ascontiguousarray(W2[128:]),
        "wp0": np.ascontiguousarray(np.concatenate([Wr1[0], Wr1[1]], 1)),
        "wp1": np.ascontiguousarray(np.concatenate([Wr1[2], Wr1[3]], 1)),
        "wp2": np.ascontiguousarray(np.concatenate([Wr1[4], Wr1[5]], 1)),
        "wp3": np.ascontiguousarray(np.concatenate([Wr1[6], Wg], 1)),  # [128, 71]
    }
    # fold b2 into the expert/gate input biases: h = relu(z~ @ Wr1 + br1') etc.
    br1_eff = br1 + np.einsum("f,kfh->kh", b2, Wr1)  # [K, RH]
    bg_eff = (bg + b2 @ Wg).astype(f)  # [K]
    out["brp0"] = np.concatenate([br1_eff[0], br1_eff[1]])[:, None].astype(f)
    out["brp1"] = np.concatenate([br1_eff[2], br1_eff[3]])[:, None].astype(f)
    out["brp2"] = np.concatenate([br1_eff[4], br1_eff[5]])[:, None].astype(f)
    out["brp3"] = np.ascontiguousarray(br1_eff[6][:, None]).astype(f)
    out["bg"] = bg_eff[:, None]

    for p in range(3):
        wb = np.zeros((128, K), f)
        wb[0:64, 2 * p] = Wr2[2 * p, :, 0]
        wb[64:128, 2 * p + 1] = Wr2[2 * p + 1, :, 0]
        out[f"wb{p}"] = wb
    wb3 = np.zeros((64, K), f)
    wb3[:, 6] = Wr2[6, :, 0]
    out["wb3"] = wb3
    return out


def _prepare_run(x, W1, b1, W2, b2, Wr1, br1, Wr2, br2, Wg, bg):
    x = np.asarray(x, np.float32)
    zero_bias = (
        not np.any(np.asarray(b1))
        and not np.any(np.asarray(b2))
        and not np.any(np.asarray(br1))
        and not np.any(np.asarray(bg))
    )

    shared = _prep_shared(W1, b1, W2, b2, Wr1, br1, Wr2, br2, Wg, bg)

    if zero_bias:
        nc = _build_fast_module(BC)
        # combined L1 stationary: rows 0..63 = W1[:, :128], 64..127 = W1[:, 128:]
        shared = {
            k: v
            for k, v in shared.items()
            if k in ("w2a", "w2b", "wp0", "wp1", "wp2", "wp3", "wb0", "wb1", "wb2", "wb3")
        }
        W1f = np.asarray(W1, np.float32)
        shared["w1ab"] = np.concatenate([W1f[:, :128], W1f[:, 128:]], 0).reshape(
            128, 128
        )
        xT = np.ascontiguousarray(x.T)  # [64, B]
    else:
        nc = _build_module(BC)
        xT = np.empty((IN + 1, B), np.float32)
        xT[:IN] = x.T
        xT[IN] = 1.0

    in_maps = []
    for i in range(NCORES):
        m = dict(shared)
        m["xT"] = np.ascontiguousarray(xT[:, i * BC : (i + 1) * BC])
        in_maps.append(m)
    return nc, in_maps


def kernel(x, domain, W1, b1, W2, b2, Wr1, br1, Wr2, br2, Wg, bg):
    global LAST_RESULTS
    domain = np.asarray(domain)
    br2 = np.asarray(br2, np.float32)

    nc, in_maps = _prepare_run(x, W1, b1, W2, b2, Wr1, br1, Wr2, br2, Wg, bg)

    res = run_bass_kernel_spmd(
        nc, in_maps, core_ids=list(range(NCORES)), trace=TRACE
    )
    LAST_RESULTS = res

    preds = np.concatenate(
        [np.asarray(r["preds_out"]).T for r in res.results], 0
    )  # [B, K]
    expw = np.concatenate([np.asarray(r["expw_out"]).T for r in res.results], 0)

    preds = preds + br2[:, 0][None, :]
    idx = np.clip(domain.astype(np.int64) - 1, 0, K - 1)
    y_hard = np.take_along_axis(preds, idx[:, None], axis=1)
    s = expw.sum(1, keepdims=True)
    weights = expw / s
    y_soft = (weights * preds).sum(1, keepdims=True)
    return (
        y_hard.astype(np.float32),
        y_soft.astype(np.float32),
        weights.astype(np.float32),
    )
